# revision 15
# baseline (speedup 1.0000x reference)
"""CrissCrossAttention Trainium2 kernel (8 NeuronCores, data-parallel).

Problem: B=4, C=256, H=W=128, 4 heads. Per head: cq=8 q/k channels, cv=64
v channels. Row attention (over W per row) + column attention (over H per
column), outputs added with the CCNet spatial-transpose quirk, then
out = gamma*attn + x.

Sharding: 16 (batch, head) pairs over 8 cores -> each core handles
batch b = core//2 and head pair p = core%2 (global heads 2p, 2p+1).
Each core reads x[b] (all 256 channels, needed by the projections) and
produces output channels [128p : 128p+128] of batch b.

Core-local pipeline (pixels indexed pix = h*128 + w):
  - qk projection -> flat row-major fr[32, h*128+w] and col-major
    fc[32, w*128+h] bf16 stores. fc comes from a second matmul pass with a
    column-ordered (strided) moving operand so both evacuations write
    near-contiguously. Bias is fused into the PSUM evacuation.
  - band-packed operand stores for the PE (matmul operands must start at
    32-aligned partitions): q/k value for row h lives at partition
    32*(h%4)+c -> the 4 rows of a group occupy distinct PE row-groups and
    their K=8 energy matmuls run concurrently via tile_position (each into
    its own PSUM bank -- concurrent row-group matmuls must not share one).
    Built from the flat stores with SBUF->SBUF DMAs (off-engine).
  - vT projection (pixel-major): vT[128w, 128h, 128c] bf16, channels =
    [64 head0 | 64 head1], Wv and bv pre-scaled by gamma on host.
  - vTc[h, w, c] = spatial transpose of vT via per-channel PE transposes
    (the DMA xbar path is a single ~26 GB/s unit -- 160 us serial stall).
  - Per row r, head hh:  eT[v,w] = k^T q (PE, K=8, 4 rows concurrent);
    pT = exp(eT) (ACT, no max subtraction -- logits are O(10));
    o2[w, 0:64] = pT.T @ vT slice; o2[w,64] = colsum via ones column
    matmul reusing the same stationary pT (softmax denominator);
    t[w, c] = o2[:, 0:64] * recip(o2[:, 64]) (DVE).
  - Column attention identical using qc/kc stores and vTc. The CCNet
    transpose aligns row-tile(row i) and col-tile(col i) elementwise on
    output row i: attn_un[j, c] = t_row(i)[j,c] + t_col(i)[j,c] (GpSimd).
  - PE-transpose attn_un (bf16) to channel-major, add residual x, DMA out.
"""

import os
import numpy as np
from contextlib import ExitStack

import concourse.bass as bass
import concourse.bacc as bacc
import concourse.tile as tile
from concourse import mybir
from concourse.masks import make_identity

F32 = mybir.dt.float32
BF16 = mybir.dt.bfloat16

B, C, H, W = 4, 256, 128, 128
PIX = H * W            # 16384
CV = 64                # v channels per head
NCORES = 8
G = 4                  # rows per attention group (= PE row-group packing)
NG = H // G            # 32 groups


def build_program():
    nc = bacc.Bacc("TRN2", target_bir_lowering=False, debug=False,
                   num_devices=NCORES)

    x_in = nc.dram_tensor("x_in", [C, PIX], F32, kind="ExternalInput")
    x_res = nc.dram_tensor("x_res", [128, PIX], F32, kind="ExternalInput")
    wqkT = nc.dram_tensor("wqkT", [C, 32], BF16, kind="ExternalInput")
    qk_bias = nc.dram_tensor("qk_bias", [32, 1], F32, kind="ExternalInput")
    wvT = nc.dram_tensor("wvT", [C, 130], BF16, kind="ExternalInput")
    vbias_row = nc.dram_tensor("vbias_row", [1, 130], BF16, kind="ExternalInput")
    out = nc.dram_tensor("out", [128, PIX], F32, kind="ExternalOutput")

    with tile.TileContext(nc) as tc, ExitStack() as ctx:
        consts = ctx.enter_context(tc.tile_pool(name="consts", bufs=1))
        persist = ctx.enter_context(tc.tile_pool(name="persist", bufs=1))

        # constants / weights
        wqa = consts.tile([128, 32], BF16, tag="wqa")
        wqb = consts.tile([128, 32], BF16, tag="wqb")
        nc.sync.dma_start(wqa, wqkT[0:128, :])
        nc.sync.dma_start(wqb, wqkT[128:256, :])
        wva = consts.tile([128, 130], BF16, tag="wva")
        wvb = consts.tile([128, 130], BF16, tag="wvb")
        nc.sync.dma_start(wva, wvT[0:128, :])
        nc.sync.dma_start(wvb, wvT[128:256, :])
        qkb = consts.tile([32, 1], F32, tag="qkb")
        nc.sync.dma_start(qkb, qk_bias[:, :])
        vbias2 = consts.tile([1, 2, 130], BF16, tag="vbias2")
        nc.sync.dma_start(vbias2[:, 0, :], vbias_row[:, :])
        nc.sync.dma_start(vbias2[:, 1, :], vbias_row[:, :])
        ones1 = consts.tile([1, 128], BF16, tag="ones1")
        nc.vector.memset(ones1, 1.0)
        identb = consts.tile([128, 128], BF16, tag="identb")
        make_identity(nc, identb)

        # persistent activations
        # band-packed operand stores: partition 32*(h%4)+c, c<8
        q_sb = persist.tile([128, 2, H // 4, W], BF16, tag="q")    # 16 KiB
        k_sb = persist.tile([128, 2, H // 4, W], BF16, tag="k")    # 16 KiB
        qc_sb = persist.tile([128, 2, W // 4, H], BF16, tag="qc")  # 16 KiB
        kc_sb = persist.tile([128, 2, W // 4, H], BF16, tag="kc")  # 16 KiB
        # pixel-major value stores, channel innermost
        vT_sb = persist.tile([128, H, 130], BF16, tag="vT")        # 32.5 KiB
        vTc_sb = persist.tile([128, W, 130], BF16, tag="vTc")      # 32.5 KiB

        # ---------------- Phase B: projections ----------------
        with (
            tc.tile_pool(name="qkflat", bufs=1) as flatpool,
            tc.tile_pool(name="xchunk", bufs=2) as xpool,
            tc.tile_pool(name="pq", bufs=2, space="PSUM") as pqpool,
            tc.tile_pool(name="pv", bufs=4, space="PSUM") as pvpool,
        ):
            fr = flatpool.tile([32, PIX], BF16, tag="fr")  # [c, h*128+w]
            fc = flatpool.tile([32, PIX], BF16, tag="fc")  # [c, w*128+h]

            CHUNK = 512  # pixels per chunk = 4 rows
            NCH = PIX // CHUNK
            for chi in range(NCH):
                c0 = chi * CHUNK
                r0 = c0 // 128
                eng = nc.sync if chi % 2 == 0 else nc.scalar
                xa = xpool.tile([128, CHUNK], F32, tag="xa")
                xb = xpool.tile([128, CHUNK], F32, tag="xb")
                eng.dma_start(xa, x_in[0:128, c0 : c0 + CHUNK])
                eng.dma_start(xb, x_in[128:256, c0 : c0 + CHUNK])
                # bf16 copies: cheaper LDWEIGHTS (FWL) for the matmuls
                xab = xpool.tile([128, CHUNK], BF16, tag="xab")
                xbb = xpool.tile([128, CHUNK], BF16, tag="xbb")
                nc.vector.tensor_copy(xab, xa[:, :])
                nc.vector.tensor_copy(xbb, xb[:, :])
                xav = xab[:, :].rearrange("p (r w) -> p r w", w=128)
                xbv = xbb[:, :].rearrange("p (r w) -> p r w", w=128)

                # qk projection, row-pixel order
                pq = pqpool.tile([32, 512], F32, tag="pq")
                nc.tensor.matmul(pq, wqa, xab[:, :], start=True, stop=False)
                nc.tensor.matmul(pq, wqb, xbb[:, :], start=False, stop=True)
                nc.vector.tensor_scalar_add(fr[:, c0 : c0 + CHUNK], pq, qkb)

                # vT projection: 2 rows per PSUM half-bank tile
                for s2 in range(2):
                    pv = pvpool.tile([128, 2, 130], F32, tag="pv")
                    for s3 in range(2):
                        srow = 2 * s2 + s3
                        # start=True only on the bank's first matmul: its
                        # has_written clear is bank-wide, and the shared
                        # bias matmul must still see row0's bits set
                        nc.tensor.matmul(pv[:, s3, :], xav[:, srow, :], wva,
                                         start=(s3 == 0), stop=False,
                                         skip_group_check=True)
                        nc.tensor.matmul(pv[:, s3, :], xbv[:, srow, :], wvb,
                                         start=False, stop=False,
                                         skip_group_check=True)
                    nc.tensor.matmul(pv[:, :, :], ones1, vbias2,
                                     start=False, stop=True,
                                     skip_group_check=True)
                    nc.scalar.copy(
                        vT_sb[:, r0 + 2 * s2 : r0 + 2 * s2 + 2, :], pv)

                # col-major flat store slices: fc[:, :, h-slice] only needs
                # fr rows h-slice -> overlap the permute with projection
                if chi % 8 == 7:
                    hs = (chi // 8) * 32
                    frv = fr[:, :].rearrange("c (h w) -> c w h", w=W)
                    fcv = fc[:, :].rearrange("c (w h) -> c w h", h=H)
                    nc.gpsimd.tensor_copy(fcv[:, :, hs : hs + 32],
                                          frv[:, :, hs : hs + 32])


            # band the flat stores (SBUF->SBUF DMA, partition moves)
            # fr [c, (hb b w)] -> q_sb[32b+c, hh, hb, w]
            for bb in range(4):
                for hh in range(2):
                    src_r = fr[:, :].rearrange(
                        "c (hb b w) -> c b hb w", b=4, w=W)
                    src_c = fc[:, :].rearrange(
                        "c (wb b h) -> c b wb h", b=4, h=H)
                    eng = nc.sync if hh == 0 else nc.scalar
                    eng.dma_start(
                        q_sb[32 * bb : 32 * bb + 8, hh, :, :],
                        src_r[8 * hh : 8 * hh + 8, bb, :, :])
                    eng.dma_start(
                        k_sb[32 * bb : 32 * bb + 8, hh, :, :],
                        src_r[16 + 8 * hh : 24 + 8 * hh, bb, :, :])
                    eng.dma_start(
                        qc_sb[32 * bb : 32 * bb + 8, hh, :, :],
                        src_c[8 * hh : 8 * hh + 8, bb, :, :])
                    eng.dma_start(
                        kc_sb[32 * bb : 32 * bb + 8, hh, :, :],
                        src_c[16 + 8 * hh : 24 + 8 * hh, bb, :, :])

        # ---------------- Phase B2: vTc via PE transposes ----------------
        # vT[w, h, c] -> vTc[h, w, c]; per channel, batched 4 per bank.
        with tc.tile_pool(name="ptr", bufs=2, space="PSUM") as ptrpool:
            for cb in range(33):
                nch = min(4, 130 - cb * 4)
                ptr = ptrpool.tile([128, 4, 128], BF16, tag="ptr")
                for cj in range(nch):
                    cch = cb * 4 + cj
                    nc.tensor.matmul(ptr[:, cj, :], vT_sb[:, :, cch], identb,
                                     start=True, stop=True, is_transpose=True)
                nc.vector.tensor_copy(
                    vTc_sb[:, :, cb * 4 : cb * 4 + nch],
                    ptr[:, 0:nch, :].rearrange("p c w -> p w c"))

        # ---------------- Phase C: attention ----------------
        with (
            tc.tile_pool(name="pe", bufs=1, space="PSUM") as pepool,
            tc.tile_pool(name="po", bufs=2, space="PSUM") as popool,
            tc.tile_pool(name="pat", bufs=2, space="PSUM") as patpool,
            tc.tile_pool(name="pt", bufs=3) as ptpool,
            tc.tile_pool(name="tt", bufs=3) as tpool,
            tc.tile_pool(name="au", bufs=2) as aupool,
            tc.tile_pool(name="rc", bufs=4) as rcpool,
            tc.tile_pool(name="io", bufs=3) as iopool,
        ):
            for g in range(NG):
                t_dir = []
                for d in range(2):  # 0 = row attention, 1 = column attention
                    qs = q_sb if d == 0 else qc_sb
                    ks = k_sb if d == 0 else kc_sb
                    vs = vT_sb if d == 0 else vTc_sb
                    til = tpool.tile([128, G, 2, CV], BF16, tag="t")
                    for hh in range(2):
                        # one PSUM bank per concurrent row-group matmul
                        pe = pepool.tile([128, G, 512], F32, tag="pe")
                        for j in range(G):
                            nc.tensor.matmul(
                                pe[:, j, 0:128],
                                ks[32 * j : 32 * j + 8, hh, g, :],
                                qs[32 * j : 32 * j + 8, hh, g, :],
                                start=True, stop=True,
                                tile_position=(32 * j, 0),
                            )
                        pT = ptpool.tile([128, G, 128], BF16, tag="pt")
                        nc.scalar.activation(
                            pT[:, 0:2, :], pe[:, 0:2, 0:128],
                            mybir.ActivationFunctionType.Exp)
                        nc.scalar.activation(
                            pT[:, 2:4, :], pe[:, 2:4, 0:128],
                            mybir.ActivationFunctionType.Exp)
                        po = popool.tile([128, G, 65], F32, tag="po")
                        for j in range(G):
                            i = g * G + j
                            nc.tensor.matmul(
                                po[:, j, :], pT[:, j, :],
                                vs[:, i, 65 * hh : 65 * hh + 65],
                                start=True, stop=True,
                            )
                        rec = rcpool.tile([128, G, 1], F32, tag="rc")
                        nc.vector.reciprocal(rec, po[:, :, 64:65])
                        nc.vector.tensor_tensor(
                            til[:, :, hh, :], po[:, :, 0:64],
                            rec.to_broadcast((128, G, CV)),
                            mybir.AluOpType.mult,
                        )
                    t_dir.append(til)
                au = aupool.tile([128, G, 128], BF16, tag="au")
                nc.gpsimd.tensor_tensor(au, t_dir[0][:, :, :, :],
                                        t_dir[1][:, :, :, :],
                                        mybir.AluOpType.add)
                pat = patpool.tile([128, G, 128], BF16, tag="pat")
                for j in range(G):
                    nc.tensor.matmul(pat[:, j, :], au[:, j, :], identb,
                                     start=True, stop=True, is_transpose=True)
                eng = nc.sync if g % 2 == 0 else nc.scalar
                xres = iopool.tile([128, G * 128], F32, tag="xres")
                eng.dma_start(xres, x_res[:, g * 512 : (g + 1) * 512])
                res = iopool.tile([128, G * 128], F32, tag="res")
                nc.vector.tensor_tensor(
                    res, pat[:, :, :].rearrange("p g w -> p (g w)"),
                    xres, mybir.AluOpType.add)
                eng.dma_start(out[:, g * 512 : (g + 1) * 512], res)

    return nc


def _prep_core_inputs(core, x, Wq, bq, Wk, bk, Wv, bv, gamma):
    b = core // 2
    p = core % 2
    g = float(np.asarray(gamma).reshape(-1)[0])
    qsl = slice(16 * p, 16 * p + 16)
    vsl = slice(128 * p, 128 * p + 128)

    import ml_dtypes
    bf = ml_dtypes.bfloat16

    wqk = np.zeros((C, 32), np.float32)
    wqk[:, 0:16] = Wq[qsl].T       # q head even(8) | q head odd(8)
    wqk[:, 16:32] = Wk[qsl].T
    wqk = wqk.astype(bf)
    qkb = np.concatenate([bq[qsl], bk[qsl]]).reshape(32, 1).astype(np.float32)

    wv_eff = (g * Wv[vsl]).astype(np.float32)     # [128, 256]
    bv_eff = (g * bv[vsl]).astype(np.float32)
    wvt = np.zeros((C, 130), np.float32)
    wvt[:, 0:64] = wv_eff[0:64].T
    wvt[:, 65:129] = wv_eff[64:128].T
    wvt = wvt.astype(bf)
    vbias = np.zeros((1, 130), np.float32)
    vbias[0, 0:64] = bv_eff[0:64]
    vbias[0, 64] = 1.0
    vbias[0, 65:129] = bv_eff[64:128]
    vbias[0, 129] = 1.0
    vbias = vbias.astype(bf)

    return {
        "x_in": np.ascontiguousarray(x[b].reshape(C, PIX), np.float32),
        "x_res": np.ascontiguousarray(x[b, vsl].reshape(128, PIX), np.float32),
        "wqkT": wqk,
        "qk_bias": qkb,
        "wvT": wvt,
        "vbias_row": vbias,
    }


_NC_CACHE = None


def _get_nc():
    global _NC_CACHE
    if _NC_CACHE is None:
        nc = build_program()
        nc.compile()
        _NC_CACHE = nc
    return _NC_CACHE


def kernel(x, Wq, bq, Wk, bk, Wv, bv, gamma, _trace=False, _trace_kwargs=None):
    from concourse.bass_utils import run_bass_kernel_spmd

    nc = _get_nc()
    in_maps = [
        _prep_core_inputs(core, x, Wq, bq, Wk, bk, Wv, bv, gamma)
        for core in range(NCORES)
    ]
    res = run_bass_kernel_spmd(
        nc, in_maps, list(range(NCORES)), trace=_trace,
        **(_trace_kwargs or {}),
    )
    outp = np.empty((B, C, H, W), np.float32)
    for core in range(NCORES):
        b, p = core // 2, core % 2
        outp[b, 128 * p : 128 * p + 128] = (
            res.results[core]["out"].reshape(128, H, W)
        )
    if _trace:
        kernel.last_results = res
    return outp
